# revision 11
# baseline (speedup 1.0000x reference)
"""GraphTransformer (TransformerConv + mean-pool) on 8 trn2 NeuronCores.

Strategy (two launches, nodes sharded 8 ways):
  Launch A (per core, 6250 nodes + pad -> 6272):
      qkv/skip = x @ W_eff + b_eff    (W_emb folded into the qkv/skip weights)
      q,k cols in fp8+DoubleRow, v/skip cols in bf16.
  Host: assign nodes to cores/tiles with degree-balanced bin-packing so each
      128-dst tile holds <=8 chunks of 128 edges; gather per-edge rows
      qg=Q[dst] (fp8), kg=K[src] (fp8), vg=V[src] (bf16); one-hot ind/indng.
  Launch B (per core, 49 dst tiles, variable chunk counts baked in):
      P        = cumsum(qg*kg) along the row            (ONE fused DVE op)
      s[e,ch,h]= P[boundary] - P[prev boundary]         (strided diff)
      w        = exp(s*scale) broadcast                 (ACT)
      wv       = w * vg                                 (DVE 2x)
      num/den += ind^T @ {wv, w}                        (TensorE, per chunk)
      mh       = num * (1/(8*den))                      (DVE, PSUM read)
      pooled  += indng^T @ mh ; pskip += indng^T @ skip (TensorE)
  Host: sum pooled head-blocks + pskip over cores, divide by graph counts.
"""

import heapq

import numpy as np
import ml_dtypes

import concourse.bass as bass
from concourse import bacc
import concourse.mybir as mybir
import concourse.tile as tile
from concourse import bass_utils
from concourse.bass import ts

BF16 = mybir.dt.bfloat16
F32 = mybir.dt.float32
FP8 = mybir.dt.float8e4
NP_BF16 = ml_dtypes.bfloat16
NP_FP8 = np.dtype(mybir.dt.np(FP8))

N, E, B = 50000, 400000, 64
IN_DIM, OUT_DIM, HEADS = 768, 64, 8
HC = HEADS * OUT_DIM  # 512
NCORES = 8
NPC = N // NCORES  # 6250 nodes per core
TILES = 49  # dst tiles per core (49*128 = 6272 >= 6250)
NPAD = TILES * 128  # 6272
MAXCH = 9  # max edge chunks (of 128) per dst tile
KCH = IN_DIM // 128  # 6 contraction chunks
W_SCALE = 256.0  # fp8 weight pre-scale (values ~1e-2 would be subnormal in e4m3)

TRACE = False
LAST_EXEC_NS = {}

_cache = {}


def _register_mul_scan():
    """Custom DVE op: out = cumsum(in0 * in1) along the free dim (fp32)."""
    from concourse.dve_spec import Spec, Src0, Src1, AluOp, scan, lower, _has_src1
    from concourse.dve_uop import DveOpSpec
    from concourse import dve_ops as dvo

    for op in dvo.OPS:
        if op.name == "MUL_SCAN_ANT":
            return op
    spec = Spec(
        body=scan(AluOp.ADD, Src0 * Src1),
        reference=lambda in0, in1, s0, s1, imm2: np.cumsum(
            in0.astype(np.float32) * in1.astype(np.float32), axis=-1
        ).astype(np.float32),
    )
    row = dvo._CUSTOM_DVE_ROW_BASE + len(dvo.OPS)
    shas = {}
    for ver in ("v3", "v4"):
        uops = lower(spec, ver=ver)
        shas[ver] = DveOpSpec(
            name="MUL_SCAN_ANT", opcode=row, uops=uops, rd1_en=_has_src1(spec)
        ).sha(ver)
    op = dvo.DveOp("MUL_SCAN_ANT", spec, subdim=False, uops_sha=shas)
    dvo.OPS.append(op)
    dvo.CUSTOM_DVE_SPECS["MUL_SCAN_ANT"] = spec
    dvo._SUB_OPCODE_FOR_NAME["MUL_SCAN_ANT"] = row
    return op


def _build_launch_a():
    # q,k (cols 0:1024) in fp8 + DoubleRow; v,skip (cols 1024:1600) in bf16.
    nc = bacc.Bacc("TRN2", debug=False, num_devices=NCORES)
    xT8 = nc.dram_tensor("xT8", [KCH * 128, NPAD], FP8, kind="ExternalInput").ap()
    xT = nc.dram_tensor("xT", [KCH * 128, NPAD], BF16, kind="ExternalInput").ap()
    w8 = nc.dram_tensor("w8", [KCH * 128, 1024], FP8, kind="ExternalInput").ap()
    w16 = nc.dram_tensor("w16", [KCH * 128, 576], BF16, kind="ExternalInput").ap()
    bqkvs = nc.dram_tensor("bqkvs", [128, 1600], BF16, kind="ExternalInput").ap()
    qk_out = nc.dram_tensor("qk_out", [NPAD, 1024], BF16, kind="ExternalOutput").ap()
    v_out = nc.dram_tensor("v_out", [NPAD, 512], BF16, kind="ExternalOutput").ap()
    skip_out = nc.dram_tensor("skip_out", [NPAD, OUT_DIM], BF16, kind="ExternalOutput").ap()

    DR = mybir.MatmulPerfMode.DoubleRow

    with tile.TileContext(nc) as tc:
        with (
            tc.tile_pool(name="const", bufs=1) as cpool,
            tc.tile_pool(name="work", bufs=3) as wpool,
            tc.tile_pool(name="psqk", bufs=2, space="PSUM") as pqk,
            tc.tile_pool(name="psvs", bufs=2, space="PSUM") as pvs,
        ):
            # x tiles split per (k-chunk, tile-group) so early matmuls only
            # wait on their own slice of the 14MB input DMA
            GT = 7  # tiles per group
            NG = TILES // GT  # 7 groups
            GW = GT * 128  # 896 cols per group
            x8_kg = [
                [cpool.tile([128, 2 * GW], FP8, name=f"x8_{s}_{g}") for g in range(NG)]
                for s in range(KCH // 2)
            ]
            x16_kg = [
                [cpool.tile([128, GW], BF16, name=f"x16_{k}_{g}") for g in range(NG)]
                for k in range(KCH)
            ]
            w8_sb = cpool.tile([128, KCH * 1024], FP8)
            w16_k = [cpool.tile([128, 576], BF16, name=f"w16_{k}") for k in range(KCH)]
            bqkvs_sb = cpool.tile([128, 1600], BF16)
            nc.sync.dma_start(bqkvs_sb[:], bqkvs[:])
            for k in range(KCH):
                nc.sync.dma_start(w8_sb[:, k * 1024:(k + 1) * 1024], w8[ts(k, 128), :])
                nc.sync.dma_start(w16_k[k][:], w16[ts(k, 128), :])
            for g in range(NG):
                for s in range(KCH // 2):
                    for i in range(2):
                        nc.sync.dma_start(
                            x8_kg[s][g][:, i * GW:(i + 1) * GW],
                            xT8[ts(2 * s + i, 128), ts(g, GW)])
                for k in range(KCH):
                    nc.sync.dma_start(x16_kg[k][g][:], xT[ts(k, 128), ts(g, GW)])

            w8v = w8_sb.rearrange("p (k n) -> p k n", k=KCH)

            for m in range(TILES):
                g, mg = divmod(m, GT)
                qk_ps = pqk.tile([128, 1024], F32, tag="qk")
                vs_ps = pvs.tile([128, 576], F32, tag="vs")
                for s in range(KCH // 2):  # fp8 DoubleRow: 2 k-chunks per pass
                    lhsT = x8_kg[s][g].rearrange(
                        "p (i m) -> p i m", i=2)[:, :, ts(mg, 128)]
                    for n0 in (0, 512):
                        nc.tensor.matmul(
                            qk_ps[:, n0:n0 + 512],
                            lhsT=lhsT,
                            rhs=w8v[:, 2 * s:2 * s + 2, n0:n0 + 512],
                            start=(s == 0),
                            stop=(s == KCH // 2 - 1),
                            perf_mode=DR,
                        )
                for k in range(KCH):  # bf16: v + skip
                    for n0, nw in ((0, 512), (512, 64)):
                        nc.tensor.matmul(
                            vs_ps[:, n0:n0 + nw],
                            lhsT=x16_kg[k][g][:, ts(mg, 128)],
                            rhs=w16_k[k][:, n0:n0 + nw],
                            start=(k == 0),
                            stop=(k == KCH - 1),
                        )
                qk_sb = wpool.tile([128, 1024], BF16, tag="qk")
                nc.vector.scalar_tensor_tensor(
                    out=qk_sb[:], in0=qk_ps[:], scalar=1.0 / W_SCALE,
                    in1=bqkvs_sb[:, :1024],
                    op0=mybir.AluOpType.mult, op1=mybir.AluOpType.add,
                )
                v_sb = wpool.tile([128, 512], BF16, tag="v")
                nc.vector.tensor_add(v_sb[:], vs_ps[:, 0:512], bqkvs_sb[:, 1024:1536])
                skip_sb = wpool.tile([128, OUT_DIM], BF16, tag="skip")
                nc.vector.tensor_add(skip_sb[:], vs_ps[:, 512:576], bqkvs_sb[:, 1536:1600])
                nc.sync.dma_start(qk_out[ts(m, 128), :], qk_sb[:])
                nc.sync.dma_start(v_out[ts(m, 128), :], v_sb[:])
                nc.sync.dma_start(skip_out[ts(m, 128), :], skip_sb[:])
    nc.compile()
    return nc


def _build_launch_b(widths):
    """widths: tuple of TILES ints, chunks per dst tile (non-increasing)."""
    mul_scan = _register_mul_scan()
    totch = sum(widths)
    offs = np.zeros(TILES + 1, np.int64)
    offs[1:] = np.cumsum(widths)

    nc = bacc.Bacc("TRN2", debug=False, num_devices=NCORES)
    qg = nc.dram_tensor("qg", [128, totch * HC], FP8, kind="ExternalInput").ap()
    kg = nc.dram_tensor("kg", [128, totch * HC], FP8, kind="ExternalInput").ap()
    vg = nc.dram_tensor("vg", [128, totch * HC], BF16, kind="ExternalInput").ap()
    ind = nc.dram_tensor("ind", [128, totch * 128], FP8, kind="ExternalInput").ap()
    skip = nc.dram_tensor("skip", [TILES, 128, OUT_DIM], BF16, kind="ExternalInput").ap()
    indng = nc.dram_tensor("indng", [TILES, 128, B], FP8, kind="ExternalInput").ap()
    pooled = nc.dram_tensor("pooled", [B, HC], F32, kind="ExternalOutput").ap()
    pskip = nc.dram_tensor("pskip", [B, OUT_DIM], F32, kind="ExternalOutput").ap()

    scale = 1.0 / np.sqrt(np.float32(OUT_DIM))
    W0 = max(widths)
    MAXFW = W0 * HC

    with tile.TileContext(nc) as tc:
        with (
            tc.tile_pool(name="io", bufs=3) as iop,
            tc.tile_pool(name="work", bufs=3) as wp,
            tc.tile_pool(name="psA", bufs=2, space="PSUM") as psA,
            tc.tile_pool(name="psB", bufs=2, space="PSUM") as psB,
            tc.tile_pool(name="psP", bufs=1, space="PSUM") as psP,
            tc.tile_pool(name="psS", bufs=1, space="PSUM") as psS,
            tc.tile_pool(name="outp", bufs=1) as outp,
        ):
            pool_ps = psP.tile([B, HC], F32)
            pskip_ps = psS.tile([B, OUT_DIM], F32)
            for t in range(TILES):
                W = widths[t]
                fw = W * HC
                nb = W * HEADS
                o = int(offs[t])
                qg_sb = iop.tile([128, MAXFW], FP8, tag="qg")
                kg_sb = iop.tile([128, MAXFW], FP8, tag="kg")
                vg_sb = iop.tile([128, MAXFW], BF16, tag="vg")
                ind_sb = iop.tile([128, W0 * 128], FP8, tag="ind")
                skip_sb = iop.tile([128, OUT_DIM], BF16, tag="skip")
                indng_sb = iop.tile([128, B], FP8, tag="indng")
                nc.sync.dma_start(qg_sb[:, :fw], qg[:, o * HC:(o + W) * HC])
                nc.sync.dma_start(kg_sb[:, :fw], kg[:, o * HC:(o + W) * HC])
                nc.sync.dma_start(vg_sb[:, :fw], vg[:, o * HC:(o + W) * HC])
                nc.sync.dma_start(ind_sb[:, :W * 128], ind[:, o * 128:(o + W) * 128])
                nc.sync.dma_start(skip_sb[:], skip[t])
                nc.sync.dma_start(indng_sb[:], indng[t])

                # prefix sums of q*k products along the whole row
                scan_f = wp.tile([128, MAXFW], F32, tag="scan")
                nc.vector._custom_dve(
                    mul_scan, out=scan_f[:, :fw], in0=qg_sb[:, :fw], in1=kg_sb[:, :fw]
                )
                # scores = diffs of prefix at 64-elem boundaries
                rpad = wp.tile([128, W0 * HEADS + 8], F32, tag="rpad")
                nc.vector.memset(rpad[:, 0:1], 0.0)
                nc.vector.tensor_copy(
                    rpad[:, 1:nb + 1],
                    scan_f[:, :fw].rearrange("p (s c) -> p s c", s=nb)[:, :, OUT_DIM - 1],
                )
                s_f = wp.tile([128, W0 * HEADS], F32, tag="s")
                nc.vector.tensor_sub(s_f[:, :nb], rpad[:, 1:nb + 1], rpad[:, 0:nb])
                # w = exp(scale * s), broadcast over the 64 head-channels
                w_bf = wp.tile([128, MAXFW], BF16, tag="w")
                nc.scalar.activation(
                    out=w_bf[:, :fw].rearrange("p (s c) -> p s c", s=nb),
                    in_=s_f[:, :nb].rearrange("p s -> p s ()").to_broadcast(
                        [128, nb, OUT_DIM]),
                    func=mybir.ActivationFunctionType.Exp,
                    scale=float(scale),
                )
                wv = wp.tile([128, MAXFW], BF16, tag="wv")
                nc.vector.tensor_mul(wv[:, :fw], vg_sb[:, :fw], w_bf[:, :fw])

                num_ps = psA.tile([128, HC], F32, tag="num")
                den_ps = psB.tile([128, HEADS], F32, tag="den")
                w4 = w_bf[:, :fw].rearrange("p (ch h c) -> p ch h c", ch=W, h=HEADS)
                for c in range(W):
                    nc.tensor.matmul(
                        num_ps[:], lhsT=ind_sb[:, ts(c, 128)], rhs=wv[:, ts(c, HC)],
                        start=(c == 0), stop=(c == W - 1),
                    )
                    nc.tensor.matmul(
                        den_ps[:], lhsT=ind_sb[:, ts(c, 128)], rhs=w4[:, c, :, 0],
                        start=(c == 0), stop=(c == W - 1),
                    )
                # mh = num / (8*den)  (mean over heads folded into rec)
                rec = wp.tile([128, HEADS], F32, tag="rec")
                nc.vector.tensor_scalar(
                    out=rec[:], in0=den_ps[:],
                    scalar1=float(HEADS), scalar2=1e-6,
                    op0=mybir.AluOpType.mult, op1=mybir.AluOpType.add,
                )
                nc.vector.reciprocal_approx_fast(rec[:], rec[:])
                mh = wp.tile([128, HC], BF16, tag="mh")
                nc.vector.tensor_mul(
                    mh.rearrange("p (h c) -> p h c", h=HEADS),
                    num_ps.rearrange("p (h c) -> p h c", h=HEADS),
                    rec.rearrange("p h -> p h ()").to_broadcast([128, HEADS, OUT_DIM]),
                )
                nc.tensor.matmul(
                    pool_ps[:], lhsT=indng_sb[:], rhs=mh[:],
                    start=(t == 0), stop=(t == TILES - 1),
                )
                nc.tensor.matmul(
                    pskip_ps[:], lhsT=indng_sb[:], rhs=skip_sb[:],
                    start=(t == 0), stop=(t == TILES - 1),
                )
            pooled_sb = outp.tile([B, HC], F32)
            nc.vector.tensor_copy(pooled_sb[:], pool_ps[:])
            nc.sync.dma_start(pooled[:], pooled_sb[:])
            pskip_sb = outp.tile([B, OUT_DIM], F32)
            nc.vector.tensor_copy(pskip_sb[:], pskip_ps[:])
            nc.sync.dma_start(pskip[:], pskip_sb[:])
    nc.compile()
    return nc


def _get_a():
    if "A" not in _cache:
        _cache["A"] = _build_launch_a()
    return _cache["A"]


def _get_b(widths):
    key = ("B", widths)
    if key not in _cache:
        _cache[key] = _build_launch_b(widths)
    return _cache[key]


def _pack_nodes(deg_dst):
    """Assign nodes to (core, tile, idx) balancing edge counts.

    Returns node2slot [N] -> global slot (core*NPAD + tile*128 + idx) and
    the per-tile chunk widths (shared across cores, non-increasing).
    """
    order = np.argsort(-deg_dst, kind="stable")
    # serpentine across cores to equalize per-core edge totals
    ranks = np.arange(N)
    blk, j = ranks // NCORES, ranks % NCORES
    core_for_rank = np.where(blk % 2 == 0, j, NCORES - 1 - j)
    node2slot = np.zeros(N, np.int64)
    widths_per_core = []
    for c in range(NCORES):
        nodes_c = order[core_for_rank == c]  # degree-descending
        # greedy: put each node in the least-loaded tile with room
        heap = [(0, 0, t) for t in range(TILES)]  # (edge load, node count, tile)
        heapq.heapify(heap)
        assign_tile = np.zeros(NPC, np.int64)
        loads = np.zeros(TILES, np.int64)
        cnts = np.zeros(TILES, np.int64)
        for i, n in enumerate(nodes_c):
            load, cnt, t2 = heapq.heappop(heap)
            assign_tile[i] = t2
            load += int(deg_dst[n])
            cnt += 1
            loads[t2] = load
            cnts[t2] = cnt
            if cnt < 128:
                heapq.heappush(heap, (load, cnt, t2))
        w_c = np.maximum(np.ceil(loads / 128).astype(np.int64), 1)
        # relabel tiles so widths are non-increasing (aligned across cores)
        tile_order = np.argsort(-(w_c * 1000000 + loads), kind="stable")
        rank_of_tile = np.empty(TILES, np.int64)
        rank_of_tile[tile_order] = np.arange(TILES)
        new_tile = rank_of_tile[assign_tile]
        # idx within tile = arrival order
        idx_within = np.zeros(NPC, np.int64)
        cnt2 = np.zeros(TILES, np.int64)
        for i in range(NPC):
            t2 = new_tile[i]
            idx_within[i] = cnt2[t2]
            cnt2[t2] += 1
        node2slot[nodes_c] = c * NPAD + new_tile * 128 + idx_within
        widths_per_core.append(w_c[tile_order])
    widths = np.max(np.stack(widths_per_core), axis=0)
    widths = np.minimum(widths, MAXCH)
    return node2slot, tuple(int(w) for w in widths)


LAST_TRACE_PATH = {}


def _ensure_hook_shim():
    import sys
    import types

    if "antenv.axon_hooks" in sys.modules:
        return
    mod = types.ModuleType("antenv.axon_hooks")
    holder = [None]
    mod.set_axon_ntff_profile_hook = lambda h: holder.__setitem__(0, h)
    mod.get_axon_ntff_profile_hook = lambda: holder[0]
    sys.modules["antenv.axon_hooks"] = mod
    import antenv

    antenv.axon_hooks = mod
    from trn_agent_boot.trn_boot import _ntff_profile_via_ctypes

    mod.set_axon_ntff_profile_hook(
        _ntff_profile_via_ctypes("/opt/axon/libaxon_pjrt.so")
    )


def _run(nc, in_maps, label):
    if not TRACE:
        res = bass_utils.run_bass_kernel_spmd(nc, in_maps, list(range(NCORES)))
        return res.results

    import glob
    import os
    import tempfile

    from concourse import bass2jax
    from concourse._compat import FishPath
    import gauge.profiler

    _ensure_hook_shim()
    import antenv.axon_hooks as hooks

    tmpdir = tempfile.mkdtemp(prefix=f"bass_{label}_")
    with hooks.get_axon_ntff_profile_hook()(tmpdir, [0]):
        results = bass2jax.run_bass_via_pjrt(nc, in_maps, n_cores=NCORES)
    exec_ns = None
    try:
        ntffs = glob.glob(os.path.join(tmpdir, "*_body*.ntff"))
        if ntffs:
            profile = gauge.profiler.Profile(
                profile_path=FishPath(tmpdir),
                kernel_dev_mode=True,
                profile_on_exit=False,
                bass_kernel=nc.m,
                offline_processing=True,
                fname="*_body*",
            )
            prs = profile.to_perfetto(model_index=(0,))
            if prs:
                exec_ns = max(p.exec_time_ns for p in prs)
                LAST_TRACE_PATH[label] = (tmpdir, [p.trace_path for p in prs])
        else:
            print(f"[{label}] no ntff files in {tmpdir}: {os.listdir(tmpdir)}")
    except Exception as e:  # profiling must never break the run
        print(f"[{label}] profile processing failed: {type(e).__name__}: {e}")
    LAST_EXEC_NS[label] = exec_ns
    return results


def kernel(x, edge_index, batch, W_emb, b_emb, Wq, bq, Wk, bk, Wv, bv, Wskip, bskip):
    x = np.asarray(x, np.float32)
    edge_index = np.asarray(edge_index)
    batch_np = np.asarray(batch)
    src = np.asarray(edge_index[0], np.int64)
    dst = np.asarray(edge_index[1], np.int64)

    deg = np.bincount(dst, minlength=N)
    node2slot, widths = _pack_nodes(deg)
    totch = sum(widths)
    ncA = _get_a()
    ncB = _get_b(widths)

    # ---- host prep for launch A: fold W_emb/b_emb into the qkv/skip weights ----
    wcat = np.concatenate(
        [np.asarray(Wq, np.float32), np.asarray(Wk, np.float32),
         np.asarray(Wv, np.float32), np.asarray(Wskip, np.float32)], axis=1
    )  # [768, 1600]
    bcat = np.concatenate(
        [np.asarray(bq, np.float32), np.asarray(bk, np.float32),
         np.asarray(bv, np.float32), np.asarray(bskip, np.float32)]
    )  # [1600]
    wemb_f = np.asarray(W_emb, np.float32)
    bemb_f = np.asarray(b_emb, np.float32)
    wqkvs_f = wemb_f @ wcat                          # [768, 1600]
    w8 = np.ascontiguousarray(wqkvs_f[:, :1024] * W_SCALE).astype(NP_FP8)
    w16 = np.ascontiguousarray(wqkvs_f[:, 1024:1600]).astype(NP_BF16)
    bqkvs = (bemb_f @ wcat + bcat).astype(np.float32)
    bqkvs_rep = np.broadcast_to(bqkvs.astype(NP_BF16), (128, 1600)).copy()

    xpad = np.zeros((NCORES * NPAD, IN_DIM), np.float32)
    xpad[node2slot] = x
    in_maps_a = []
    for c in range(NCORES):
        xT = np.ascontiguousarray(xpad[c * NPAD:(c + 1) * NPAD].T)  # [768, 6272]
        in_maps_a.append({
            "xT8": xT.astype(NP_FP8), "xT": xT.astype(NP_BF16),
            "w8": w8, "w16": w16, "bqkvs": bqkvs_rep,
        })
    res_a = _run(ncA, in_maps_a, "A")

    # ---- host mid: slot-ordered Q,K,V and edge-sorted gathers ----
    QK8 = np.concatenate([res_a[c]["qk_out"] for c in range(NCORES)]).astype(NP_FP8)
    V = np.concatenate([res_a[c]["v_out"] for c in range(NCORES)])  # bf16 [8*NPAD,512]

    dslot = node2slot[dst]
    tile_g = dslot // 128  # global tile id: core*TILES + tile
    dloc = dslot % 128
    order = np.argsort(tile_g, kind="stable")
    tg_s, src_s, dloc_s, dslot_s = tile_g[order], src[order], dloc[order], dslot[order]
    ntile = NCORES * TILES
    counts = np.bincount(tg_s, minlength=ntile)
    wid_g = np.tile(np.asarray(widths, np.int64), NCORES)
    cap_g = wid_g * 128
    if np.any(counts > cap_g):
        raise RuntimeError("tile capacity exceeded after packing")
    # edge-slot base per global tile in the variable-width flat layout
    tots = totch * 128  # slots per core
    cumw = np.zeros(TILES + 1, np.int64)
    cumw[1:] = np.cumsum(np.asarray(widths, np.int64))
    ebase = (tg_s // TILES) * tots + cumw[tg_s % TILES] * 128
    starts = np.zeros(ntile, np.int64)
    starts[1:] = np.cumsum(counts)[:-1]
    pos = np.arange(E) - starts[tg_s]
    rows = ebase + pos

    nslot_t = NCORES * tots
    srcslot_pad = np.zeros(nslot_t, np.int64)
    srcslot_pad[rows] = node2slot[src_s]
    dloc_pad = np.full(nslot_t, -1, np.int64)
    dloc_pad[rows] = dloc_s
    dslot_pad = np.zeros(nslot_t, np.int64)
    dslot_pad[rows] = dslot_s

    def tileize(a):  # per core [tots, D] -> [128, totch*D], chunk-major cols
        d = a.shape[1]
        return np.ascontiguousarray(
            a.reshape(totch, 128, d).transpose(1, 0, 2).reshape(128, totch * d)
        )

    qg_f = QK8[dslot_pad, 0:512].reshape(NCORES, tots, 512)
    kg_f = QK8[srcslot_pad, 512:1024].reshape(NCORES, tots, 512)
    vg_f = V[srcslot_pad].reshape(NCORES, tots, 512)
    ind_f = (dloc_pad[:, None] == np.arange(128)[None, :]).astype(NP_FP8).reshape(
        NCORES, tots, 128)

    batch_pad = np.full(NCORES * NPAD, -1, np.int64)
    batch_pad[node2slot] = batch_np
    indng = (batch_pad[:, None] == np.arange(B)[None, :]).astype(NP_FP8)
    indng = indng.reshape(NCORES, TILES, 128, B)

    in_maps_b = []
    for c in range(NCORES):
        in_maps_b.append({
            "qg": tileize(qg_f[c]), "kg": tileize(kg_f[c]),
            "vg": tileize(vg_f[c]), "ind": tileize(ind_f[c]),
            "skip": np.ascontiguousarray(
                res_a[c]["skip_out"].reshape(TILES, 128, OUT_DIM)),
            "indng": indng[c],
        })
    res_b = _run(ncB, in_maps_b, "B")

    pooled = np.zeros((B, OUT_DIM), np.float64)
    for c in range(NCORES):
        ph = res_b[c]["pooled"].astype(np.float64)  # [B, 512]
        pooled += ph.reshape(B, HEADS, OUT_DIM).sum(axis=1)
        pooled += res_b[c]["pskip"].astype(np.float64)
    cnt = np.bincount(batch_np, minlength=B).astype(np.float64)
    pooled /= np.maximum(cnt, 1.0)[:, None]
    return pooled.astype(np.float32)


# revision 12
# speedup vs baseline: 1.2205x; 1.2205x over previous
"""GraphTransformer (TransformerConv + mean-pool) on 8 trn2 NeuronCores.

Strategy (two launches, nodes sharded 8 ways):
  Launch A (per core, 6250 nodes + pad -> 6272):
      qkv/skip = x @ W_eff + b_eff    (W_emb folded into the qkv/skip weights)
      q,k cols in fp8+DoubleRow, v/skip cols in bf16.
  Host: assign nodes to cores/tiles with degree-balanced bin-packing so each
      128-dst tile holds <=8 chunks of 128 edges; gather per-edge rows
      qg=Q[dst] (fp8), kg=K[src] (fp8), vg=V[src] (bf16); one-hot ind/indng.
  Launch B (per core, 49 dst tiles, variable chunk counts baked in):
      P        = cumsum(qg*kg) along the row            (ONE fused DVE op)
      s[e,ch,h]= P[boundary] - P[prev boundary]         (strided diff)
      w        = exp(s*scale) broadcast                 (ACT)
      wv       = w * vg                                 (DVE 2x)
      num/den += ind^T @ {wv, w}                        (TensorE, per chunk)
      mh       = num * (1/(8*den))                      (DVE, PSUM read)
      pooled  += indng^T @ mh ; pskip += indng^T @ skip (TensorE)
  Host: sum pooled head-blocks + pskip over cores, divide by graph counts.
"""

import heapq

import numpy as np
import ml_dtypes

import concourse.bass as bass
from concourse import bacc
import concourse.mybir as mybir
import concourse.tile as tile
from concourse import bass_utils
from concourse.bass import ts

BF16 = mybir.dt.bfloat16
F32 = mybir.dt.float32
FP8 = mybir.dt.float8e4
NP_BF16 = ml_dtypes.bfloat16
NP_FP8 = np.dtype(mybir.dt.np(FP8))

N, E, B = 50000, 400000, 64
IN_DIM, OUT_DIM, HEADS = 768, 64, 8
HC = HEADS * OUT_DIM  # 512
NCORES = 8
NPC = N // NCORES  # 6250 nodes per core
TILES = 49  # dst tiles per core (49*128 = 6272 >= 6250)
NPAD = TILES * 128  # 6272
MAXCH = 9  # max edge chunks (of 128) per dst tile
KCH = IN_DIM // 128  # 6 contraction chunks
W_SCALE = 256.0  # fp8 weight pre-scale (values ~1e-2 would be subnormal in e4m3)

TRACE = False
LAST_EXEC_NS = {}

_cache = {}


def _register_mul_scan():
    """Custom DVE op: out = cumsum(in0 * in1) along the free dim (fp32)."""
    from concourse.dve_spec import Spec, Src0, Src1, AluOp, scan, lower, _has_src1
    from concourse.dve_uop import DveOpSpec
    from concourse import dve_ops as dvo

    for op in dvo.OPS:
        if op.name == "MUL_SCAN_ANT":
            return op
    spec = Spec(
        body=scan(AluOp.ADD, Src0 * Src1),
        reference=lambda in0, in1, s0, s1, imm2: np.cumsum(
            in0.astype(np.float32) * in1.astype(np.float32), axis=-1
        ).astype(np.float32),
    )
    row = dvo._CUSTOM_DVE_ROW_BASE + len(dvo.OPS)
    shas = {}
    for ver in ("v3", "v4"):
        uops = lower(spec, ver=ver)
        shas[ver] = DveOpSpec(
            name="MUL_SCAN_ANT", opcode=row, uops=uops, rd1_en=_has_src1(spec)
        ).sha(ver)
    op = dvo.DveOp("MUL_SCAN_ANT", spec, subdim=False, uops_sha=shas)
    dvo.OPS.append(op)
    dvo.CUSTOM_DVE_SPECS["MUL_SCAN_ANT"] = spec
    dvo._SUB_OPCODE_FOR_NAME["MUL_SCAN_ANT"] = row
    return op


def _build_launch_a():
    # q,k (cols 0:1024) in fp8 + DoubleRow; v,skip (cols 1024:1600) in bf16.
    nc = bacc.Bacc("TRN2", debug=False, num_devices=NCORES)
    xT8 = nc.dram_tensor("xT8", [KCH * 128, NPAD], FP8, kind="ExternalInput").ap()
    xT = nc.dram_tensor("xT", [KCH * 128, NPAD], BF16, kind="ExternalInput").ap()
    w8 = nc.dram_tensor("w8", [KCH * 128, 1024], FP8, kind="ExternalInput").ap()
    w16 = nc.dram_tensor("w16", [KCH * 128, 576], BF16, kind="ExternalInput").ap()
    bqkvs = nc.dram_tensor("bqkvs", [128, 1600], BF16, kind="ExternalInput").ap()
    qk_out = nc.dram_tensor("qk_out", [NPAD, 1024], BF16, kind="ExternalOutput").ap()
    v_out = nc.dram_tensor("v_out", [NPAD, 512], BF16, kind="ExternalOutput").ap()
    skip_out = nc.dram_tensor("skip_out", [NPAD, OUT_DIM], BF16, kind="ExternalOutput").ap()

    DR = mybir.MatmulPerfMode.DoubleRow

    with tile.TileContext(nc) as tc:
        with (
            tc.tile_pool(name="const", bufs=1) as cpool,
            tc.tile_pool(name="work", bufs=3) as wpool,
            tc.tile_pool(name="psqk", bufs=2, space="PSUM") as pqk,
            tc.tile_pool(name="psvs", bufs=2, space="PSUM") as pvs,
        ):
            # fp8 operands in single tiles (DoubleRow APs span two k-chunks);
            # bf16 x per-k so early matmuls only wait on their own DMA chunk
            xT8_sb = cpool.tile([128, KCH * NPAD], FP8)
            w8_sb = cpool.tile([128, KCH * 1024], FP8)
            x16_k = [cpool.tile([128, NPAD], BF16, name=f"x16_{k}") for k in range(KCH)]
            w16_k = [cpool.tile([128, 576], BF16, name=f"w16_{k}") for k in range(KCH)]
            bqkvs_sb = cpool.tile([128, 1600], BF16)
            nc.sync.dma_start(bqkvs_sb[:], bqkvs[:])
            for k in range(KCH):
                nc.sync.dma_start(w8_sb[:, k * 1024:(k + 1) * 1024], w8[ts(k, 128), :])
                nc.sync.dma_start(w16_k[k][:], w16[ts(k, 128), :])
                nc.sync.dma_start(xT8_sb[:, k * NPAD:(k + 1) * NPAD], xT8[ts(k, 128), :])
                nc.sync.dma_start(x16_k[k][:], xT[ts(k, 128), :])

            x8v = xT8_sb.rearrange("p (k m) -> p k m", k=KCH)
            w8v = w8_sb.rearrange("p (k n) -> p k n", k=KCH)

            for m in range(TILES):
                qk_ps = pqk.tile([128, 1024], F32, tag="qk")
                vs_ps = pvs.tile([128, 576], F32, tag="vs")
                for s in range(KCH // 2):  # fp8 DoubleRow: 2 k-chunks per pass
                    for n0 in (0, 512):
                        nc.tensor.matmul(
                            qk_ps[:, n0:n0 + 512],
                            lhsT=x8v[:, 2 * s:2 * s + 2, ts(m, 128)],
                            rhs=w8v[:, 2 * s:2 * s + 2, n0:n0 + 512],
                            start=(s == 0),
                            stop=(s == KCH // 2 - 1),
                            perf_mode=DR,
                        )
                for k in range(KCH):  # bf16: v + skip
                    for n0, nw in ((0, 512), (512, 64)):
                        nc.tensor.matmul(
                            vs_ps[:, n0:n0 + nw],
                            lhsT=x16_k[k][:, ts(m, 128)],
                            rhs=w16_k[k][:, n0:n0 + nw],
                            start=(k == 0),
                            stop=(k == KCH - 1),
                        )
                qk_sb = wpool.tile([128, 1024], BF16, tag="qk")
                nc.vector.scalar_tensor_tensor(
                    out=qk_sb[:], in0=qk_ps[:], scalar=1.0 / W_SCALE,
                    in1=bqkvs_sb[:, :1024],
                    op0=mybir.AluOpType.mult, op1=mybir.AluOpType.add,
                )
                v_sb = wpool.tile([128, 512], BF16, tag="v")
                nc.vector.tensor_add(v_sb[:], vs_ps[:, 0:512], bqkvs_sb[:, 1024:1536])
                skip_sb = wpool.tile([128, OUT_DIM], BF16, tag="skip")
                nc.vector.tensor_add(skip_sb[:], vs_ps[:, 512:576], bqkvs_sb[:, 1536:1600])
                nc.sync.dma_start(qk_out[ts(m, 128), :], qk_sb[:])
                nc.sync.dma_start(v_out[ts(m, 128), :], v_sb[:])
                nc.sync.dma_start(skip_out[ts(m, 128), :], skip_sb[:])
    nc.compile()
    return nc


def _build_launch_b(widths):
    """widths: tuple of TILES ints, chunks per dst tile (non-increasing)."""
    mul_scan = _register_mul_scan()
    totch = sum(widths)
    offs = np.zeros(TILES + 1, np.int64)
    offs[1:] = np.cumsum(widths)

    nc = bacc.Bacc("TRN2", debug=False, num_devices=NCORES)
    qg = nc.dram_tensor("qg", [128, totch * HC], FP8, kind="ExternalInput").ap()
    kg = nc.dram_tensor("kg", [128, totch * HC], FP8, kind="ExternalInput").ap()
    vg = nc.dram_tensor("vg", [128, totch * HC], BF16, kind="ExternalInput").ap()
    ind = nc.dram_tensor("ind", [128, totch * 128], FP8, kind="ExternalInput").ap()
    skip = nc.dram_tensor("skip", [TILES, 128, OUT_DIM], BF16, kind="ExternalInput").ap()
    indng = nc.dram_tensor("indng", [TILES, 128, B], FP8, kind="ExternalInput").ap()
    pooled = nc.dram_tensor("pooled", [B, HC], F32, kind="ExternalOutput").ap()
    pskip = nc.dram_tensor("pskip", [B, OUT_DIM], F32, kind="ExternalOutput").ap()

    scale = 1.0 / np.sqrt(np.float32(OUT_DIM))
    W0 = max(widths)
    MAXFW = W0 * HC

    with tile.TileContext(nc) as tc:
        with (
            tc.tile_pool(name="io", bufs=3) as iop,
            tc.tile_pool(name="work", bufs=2) as wp,
            tc.tile_pool(name="psA", bufs=2, space="PSUM") as psA,
            tc.tile_pool(name="psB", bufs=2, space="PSUM") as psB,
            tc.tile_pool(name="psP", bufs=1, space="PSUM") as psP,
            tc.tile_pool(name="psS", bufs=1, space="PSUM") as psS,
            tc.tile_pool(name="outp", bufs=1) as outp,
        ):
            pool_ps = psP.tile([B, HC], F32)
            pskip_ps = psS.tile([B, OUT_DIM], F32)
            for t in range(TILES):
                W = widths[t]
                fw = W * HC
                nb = W * HEADS
                o = int(offs[t])
                qg_sb = iop.tile([128, MAXFW], FP8, tag="qg")
                kg_sb = iop.tile([128, MAXFW], FP8, tag="kg")
                vg_sb = iop.tile([128, MAXFW], BF16, tag="vg")
                ind_sb = iop.tile([128, W0 * 128], FP8, tag="ind")
                skip_sb = iop.tile([128, OUT_DIM], BF16, tag="skip")
                indng_sb = iop.tile([128, B], FP8, tag="indng")
                nc.sync.dma_start(qg_sb[:, :fw], qg[:, o * HC:(o + W) * HC])
                nc.sync.dma_start(kg_sb[:, :fw], kg[:, o * HC:(o + W) * HC])
                nc.sync.dma_start(vg_sb[:, :fw], vg[:, o * HC:(o + W) * HC])
                nc.sync.dma_start(ind_sb[:, :W * 128], ind[:, o * 128:(o + W) * 128])
                nc.sync.dma_start(skip_sb[:], skip[t])
                nc.sync.dma_start(indng_sb[:], indng[t])

                # prefix sums of q*k products along the whole row
                scan_f = wp.tile([128, MAXFW], F32, tag="scan")
                nc.vector._custom_dve(
                    mul_scan, out=scan_f[:, :fw], in0=qg_sb[:, :fw], in1=kg_sb[:, :fw]
                )
                # scores = diffs of prefix at 64-elem boundaries
                rpad = wp.tile([128, W0 * HEADS + 8], F32, tag="rpad")
                nc.vector.memset(rpad[:, 0:1], 0.0)
                nc.vector.tensor_copy(
                    rpad[:, 1:nb + 1],
                    scan_f[:, :fw].rearrange("p (s c) -> p s c", s=nb)[:, :, OUT_DIM - 1],
                )
                s_f = wp.tile([128, W0 * HEADS], F32, tag="s")
                nc.vector.tensor_sub(s_f[:, :nb], rpad[:, 1:nb + 1], rpad[:, 0:nb])
                # w = exp(scale * s), broadcast over the 64 head-channels
                w_bf = wp.tile([128, MAXFW], BF16, tag="w")
                nc.scalar.activation(
                    out=w_bf[:, :fw].rearrange("p (s c) -> p s c", s=nb),
                    in_=s_f[:, :nb].rearrange("p s -> p s ()").to_broadcast(
                        [128, nb, OUT_DIM]),
                    func=mybir.ActivationFunctionType.Exp,
                    scale=float(scale),
                )
                wv = wp.tile([128, MAXFW], BF16, tag="wv")
                nc.vector.tensor_mul(wv[:, :fw], vg_sb[:, :fw], w_bf[:, :fw])

                num_ps = psA.tile([128, HC], F32, tag="num")
                den_ps = psB.tile([128, HEADS], F32, tag="den")
                w4 = w_bf[:, :fw].rearrange("p (ch h c) -> p ch h c", ch=W, h=HEADS)
                for c in range(W):
                    nc.tensor.matmul(
                        num_ps[:], lhsT=ind_sb[:, ts(c, 128)], rhs=wv[:, ts(c, HC)],
                        start=(c == 0), stop=(c == W - 1),
                    )
                    nc.tensor.matmul(
                        den_ps[:], lhsT=ind_sb[:, ts(c, 128)], rhs=w4[:, c, :, 0],
                        start=(c == 0), stop=(c == W - 1),
                    )
                # mh = num / (8*den)  (mean over heads folded into rec)
                rec = wp.tile([128, HEADS], F32, tag="rec")
                nc.vector.tensor_scalar(
                    out=rec[:], in0=den_ps[:],
                    scalar1=float(HEADS), scalar2=1e-6,
                    op0=mybir.AluOpType.mult, op1=mybir.AluOpType.add,
                )
                nc.vector.reciprocal_approx_fast(rec[:], rec[:])
                mh = wp.tile([128, HC], BF16, tag="mh")
                nc.vector.tensor_mul(
                    mh.rearrange("p (h c) -> p h c", h=HEADS),
                    num_ps.rearrange("p (h c) -> p h c", h=HEADS),
                    rec.rearrange("p h -> p h ()").to_broadcast([128, HEADS, OUT_DIM]),
                )
                nc.tensor.matmul(
                    pool_ps[:], lhsT=indng_sb[:], rhs=mh[:],
                    start=(t == 0), stop=(t == TILES - 1),
                )
                nc.tensor.matmul(
                    pskip_ps[:], lhsT=indng_sb[:], rhs=skip_sb[:],
                    start=(t == 0), stop=(t == TILES - 1),
                )
            pooled_sb = outp.tile([B, HC], F32)
            nc.vector.tensor_copy(pooled_sb[:], pool_ps[:])
            nc.sync.dma_start(pooled[:], pooled_sb[:])
            pskip_sb = outp.tile([B, OUT_DIM], F32)
            nc.vector.tensor_copy(pskip_sb[:], pskip_ps[:])
            nc.sync.dma_start(pskip[:], pskip_sb[:])
    nc.compile()
    return nc


def _get_a():
    if "A" not in _cache:
        _cache["A"] = _build_launch_a()
    return _cache["A"]


def _get_b(widths):
    key = ("B", widths)
    if key not in _cache:
        _cache[key] = _build_launch_b(widths)
    return _cache[key]


def _pack_nodes(deg_dst):
    """Assign nodes to (core, tile, idx) balancing edge counts.

    Returns node2slot [N] -> global slot (core*NPAD + tile*128 + idx) and
    the per-tile chunk widths (shared across cores, non-increasing).
    """
    order = np.argsort(-deg_dst, kind="stable")
    # serpentine across cores to equalize per-core edge totals
    ranks = np.arange(N)
    blk, j = ranks // NCORES, ranks % NCORES
    core_for_rank = np.where(blk % 2 == 0, j, NCORES - 1 - j)
    node2slot = np.zeros(N, np.int64)
    widths_per_core = []
    for c in range(NCORES):
        nodes_c = order[core_for_rank == c]  # degree-descending
        # greedy: put each node in the least-loaded tile with room
        heap = [(0, 0, t) for t in range(TILES)]  # (edge load, node count, tile)
        heapq.heapify(heap)
        assign_tile = np.zeros(NPC, np.int64)
        loads = np.zeros(TILES, np.int64)
        cnts = np.zeros(TILES, np.int64)
        for i, n in enumerate(nodes_c):
            load, cnt, t2 = heapq.heappop(heap)
            assign_tile[i] = t2
            load += int(deg_dst[n])
            cnt += 1
            loads[t2] = load
            cnts[t2] = cnt
            if cnt < 128:
                heapq.heappush(heap, (load, cnt, t2))
        w_c = np.maximum(np.ceil(loads / 128).astype(np.int64), 1)
        # relabel tiles so widths are non-increasing (aligned across cores)
        tile_order = np.argsort(-(w_c * 1000000 + loads), kind="stable")
        rank_of_tile = np.empty(TILES, np.int64)
        rank_of_tile[tile_order] = np.arange(TILES)
        new_tile = rank_of_tile[assign_tile]
        # idx within tile = arrival order
        idx_within = np.zeros(NPC, np.int64)
        cnt2 = np.zeros(TILES, np.int64)
        for i in range(NPC):
            t2 = new_tile[i]
            idx_within[i] = cnt2[t2]
            cnt2[t2] += 1
        node2slot[nodes_c] = c * NPAD + new_tile * 128 + idx_within
        widths_per_core.append(w_c[tile_order])
    widths = np.max(np.stack(widths_per_core), axis=0)
    widths = np.minimum(widths, MAXCH)
    return node2slot, tuple(int(w) for w in widths)


LAST_TRACE_PATH = {}


def _ensure_hook_shim():
    import sys
    import types

    if "antenv.axon_hooks" in sys.modules:
        return
    mod = types.ModuleType("antenv.axon_hooks")
    holder = [None]
    mod.set_axon_ntff_profile_hook = lambda h: holder.__setitem__(0, h)
    mod.get_axon_ntff_profile_hook = lambda: holder[0]
    sys.modules["antenv.axon_hooks"] = mod
    import antenv

    antenv.axon_hooks = mod
    from trn_agent_boot.trn_boot import _ntff_profile_via_ctypes

    mod.set_axon_ntff_profile_hook(
        _ntff_profile_via_ctypes("/opt/axon/libaxon_pjrt.so")
    )


def _run(nc, in_maps, label):
    if not TRACE:
        res = bass_utils.run_bass_kernel_spmd(nc, in_maps, list(range(NCORES)))
        return res.results

    import glob
    import os
    import tempfile

    from concourse import bass2jax
    from concourse._compat import FishPath
    import gauge.profiler

    _ensure_hook_shim()
    import antenv.axon_hooks as hooks

    tmpdir = tempfile.mkdtemp(prefix=f"bass_{label}_")
    with hooks.get_axon_ntff_profile_hook()(tmpdir, [0]):
        results = bass2jax.run_bass_via_pjrt(nc, in_maps, n_cores=NCORES)
    exec_ns = None
    try:
        ntffs = glob.glob(os.path.join(tmpdir, "*_body*.ntff"))
        if ntffs:
            profile = gauge.profiler.Profile(
                profile_path=FishPath(tmpdir),
                kernel_dev_mode=True,
                profile_on_exit=False,
                bass_kernel=nc.m,
                offline_processing=True,
                fname="*_body*",
            )
            prs = profile.to_perfetto(model_index=(0,))
            if prs:
                exec_ns = max(p.exec_time_ns for p in prs)
                LAST_TRACE_PATH[label] = (tmpdir, [p.trace_path for p in prs])
        else:
            print(f"[{label}] no ntff files in {tmpdir}: {os.listdir(tmpdir)}")
    except Exception as e:  # profiling must never break the run
        print(f"[{label}] profile processing failed: {type(e).__name__}: {e}")
    LAST_EXEC_NS[label] = exec_ns
    return results


def kernel(x, edge_index, batch, W_emb, b_emb, Wq, bq, Wk, bk, Wv, bv, Wskip, bskip):
    x = np.asarray(x, np.float32)
    edge_index = np.asarray(edge_index)
    batch_np = np.asarray(batch)
    src = np.asarray(edge_index[0], np.int64)
    dst = np.asarray(edge_index[1], np.int64)

    deg = np.bincount(dst, minlength=N)
    node2slot, widths = _pack_nodes(deg)
    totch = sum(widths)
    ncA = _get_a()
    ncB = _get_b(widths)

    # ---- host prep for launch A: fold W_emb/b_emb into the qkv/skip weights ----
    wcat = np.concatenate(
        [np.asarray(Wq, np.float32), np.asarray(Wk, np.float32),
         np.asarray(Wv, np.float32), np.asarray(Wskip, np.float32)], axis=1
    )  # [768, 1600]
    bcat = np.concatenate(
        [np.asarray(bq, np.float32), np.asarray(bk, np.float32),
         np.asarray(bv, np.float32), np.asarray(bskip, np.float32)]
    )  # [1600]
    wemb_f = np.asarray(W_emb, np.float32)
    bemb_f = np.asarray(b_emb, np.float32)
    wqkvs_f = wemb_f @ wcat                          # [768, 1600]
    w8 = np.ascontiguousarray(wqkvs_f[:, :1024] * W_SCALE).astype(NP_FP8)
    w16 = np.ascontiguousarray(wqkvs_f[:, 1024:1600]).astype(NP_BF16)
    bqkvs = (bemb_f @ wcat + bcat).astype(np.float32)
    bqkvs_rep = np.broadcast_to(bqkvs.astype(NP_BF16), (128, 1600)).copy()

    xpad = np.zeros((NCORES * NPAD, IN_DIM), np.float32)
    xpad[node2slot] = x
    in_maps_a = []
    for c in range(NCORES):
        xT = np.ascontiguousarray(xpad[c * NPAD:(c + 1) * NPAD].T)  # [768, 6272]
        in_maps_a.append({
            "xT8": xT.astype(NP_FP8), "xT": xT.astype(NP_BF16),
            "w8": w8, "w16": w16, "bqkvs": bqkvs_rep,
        })
    res_a = _run(ncA, in_maps_a, "A")

    # ---- host mid: slot-ordered Q,K,V and edge-sorted gathers ----
    QK8 = np.concatenate([res_a[c]["qk_out"] for c in range(NCORES)]).astype(NP_FP8)
    V = np.concatenate([res_a[c]["v_out"] for c in range(NCORES)])  # bf16 [8*NPAD,512]

    dslot = node2slot[dst]
    tile_g = dslot // 128  # global tile id: core*TILES + tile
    dloc = dslot % 128
    order = np.argsort(tile_g, kind="stable")
    tg_s, src_s, dloc_s, dslot_s = tile_g[order], src[order], dloc[order], dslot[order]
    ntile = NCORES * TILES
    counts = np.bincount(tg_s, minlength=ntile)
    wid_g = np.tile(np.asarray(widths, np.int64), NCORES)
    cap_g = wid_g * 128
    if np.any(counts > cap_g):
        raise RuntimeError("tile capacity exceeded after packing")
    # edge-slot base per global tile in the variable-width flat layout
    tots = totch * 128  # slots per core
    cumw = np.zeros(TILES + 1, np.int64)
    cumw[1:] = np.cumsum(np.asarray(widths, np.int64))
    ebase = (tg_s // TILES) * tots + cumw[tg_s % TILES] * 128
    starts = np.zeros(ntile, np.int64)
    starts[1:] = np.cumsum(counts)[:-1]
    pos = np.arange(E) - starts[tg_s]
    rows = ebase + pos

    nslot_t = NCORES * tots
    srcslot_pad = np.zeros(nslot_t, np.int64)
    srcslot_pad[rows] = node2slot[src_s]
    dloc_pad = np.full(nslot_t, -1, np.int64)
    dloc_pad[rows] = dloc_s
    dslot_pad = np.zeros(nslot_t, np.int64)
    dslot_pad[rows] = dslot_s

    def tileize(a):  # per core [tots, D] -> [128, totch*D], chunk-major cols
        d = a.shape[1]
        return np.ascontiguousarray(
            a.reshape(totch, 128, d).transpose(1, 0, 2).reshape(128, totch * d)
        )

    qg_f = QK8[dslot_pad, 0:512].reshape(NCORES, tots, 512)
    kg_f = QK8[srcslot_pad, 512:1024].reshape(NCORES, tots, 512)
    vg_f = V[srcslot_pad].reshape(NCORES, tots, 512)
    ind_f = (dloc_pad[:, None] == np.arange(128)[None, :]).astype(NP_FP8).reshape(
        NCORES, tots, 128)

    batch_pad = np.full(NCORES * NPAD, -1, np.int64)
    batch_pad[node2slot] = batch_np
    indng = (batch_pad[:, None] == np.arange(B)[None, :]).astype(NP_FP8)
    indng = indng.reshape(NCORES, TILES, 128, B)

    in_maps_b = []
    for c in range(NCORES):
        in_maps_b.append({
            "qg": tileize(qg_f[c]), "kg": tileize(kg_f[c]),
            "vg": tileize(vg_f[c]), "ind": tileize(ind_f[c]),
            "skip": np.ascontiguousarray(
                res_a[c]["skip_out"].reshape(TILES, 128, OUT_DIM)),
            "indng": indng[c],
        })
    res_b = _run(ncB, in_maps_b, "B")

    pooled = np.zeros((B, OUT_DIM), np.float64)
    for c in range(NCORES):
        ph = res_b[c]["pooled"].astype(np.float64)  # [B, 512]
        pooled += ph.reshape(B, HEADS, OUT_DIM).sum(axis=1)
        pooled += res_b[c]["pskip"].astype(np.float64)
    cnt = np.bincount(batch_np, minlength=B).astype(np.float64)
    pooled /= np.maximum(cnt, 1.0)[:, None]
    return pooled.astype(np.float32)


# revision 13
# speedup vs baseline: 1.2244x; 1.0032x over previous
"""GraphTransformer (TransformerConv + mean-pool) on 8 trn2 NeuronCores.

Strategy (two launches, nodes sharded 8 ways):
  Launch A (per core, 6250 nodes + pad -> 6272):
      qkv/skip = x @ W_eff + b_eff    (W_emb folded into the qkv/skip weights)
      q,k cols in fp8+DoubleRow, v/skip cols in bf16.
  Host: assign nodes to cores/tiles with degree-balanced bin-packing so each
      128-dst tile holds <=8 chunks of 128 edges; gather per-edge rows
      qg=Q[dst] (fp8), kg=K[src] (fp8), vg=V[src] (bf16); one-hot ind/indng.
  Launch B (per core, 49 dst tiles, variable chunk counts baked in):
      P        = cumsum(qg*kg) along the row            (ONE fused DVE op)
      s[e,ch,h]= P[boundary] - P[prev boundary]         (strided diff)
      w        = exp(s*scale) broadcast                 (ACT)
      wv       = w * vg                                 (DVE 2x)
      num/den += ind^T @ {wv, w}                        (TensorE, per chunk)
      mh       = num * (1/(8*den))                      (DVE, PSUM read)
      pooled  += indng^T @ mh ; pskip += indng^T @ skip (TensorE)
  Host: sum pooled head-blocks + pskip over cores, divide by graph counts.
"""

import heapq

import numpy as np
import ml_dtypes

import concourse.bass as bass
from concourse import bacc
import concourse.mybir as mybir
import concourse.tile as tile
from concourse import bass_utils
from concourse.bass import ts

BF16 = mybir.dt.bfloat16
F32 = mybir.dt.float32
FP8 = mybir.dt.float8e4
NP_BF16 = ml_dtypes.bfloat16
NP_FP8 = np.dtype(mybir.dt.np(FP8))

N, E, B = 50000, 400000, 64
IN_DIM, OUT_DIM, HEADS = 768, 64, 8
HC = HEADS * OUT_DIM  # 512
NCORES = 8
NPC = N // NCORES  # 6250 nodes per core
TILES = 49  # dst tiles per core (49*128 = 6272 >= 6250)
NPAD = TILES * 128  # 6272
MAXCH = 9  # max edge chunks (of 128) per dst tile
KCH = IN_DIM // 128  # 6 contraction chunks
W_SCALE = 256.0  # fp8 weight pre-scale (values ~1e-2 would be subnormal in e4m3)

TRACE = False
LAST_EXEC_NS = {}

_cache = {}


def _register_mul_scan():
    """Custom DVE op: out = cumsum(in0 * in1) along the free dim (fp32)."""
    from concourse.dve_spec import Spec, Src0, Src1, AluOp, scan, lower, _has_src1
    from concourse.dve_uop import DveOpSpec
    from concourse import dve_ops as dvo

    for op in dvo.OPS:
        if op.name == "MUL_SCAN_ANT":
            return op
    spec = Spec(
        body=scan(AluOp.ADD, Src0 * Src1),
        reference=lambda in0, in1, s0, s1, imm2: np.cumsum(
            in0.astype(np.float32) * in1.astype(np.float32), axis=-1
        ).astype(np.float32),
    )
    row = dvo._CUSTOM_DVE_ROW_BASE + len(dvo.OPS)
    shas = {}
    for ver in ("v3", "v4"):
        uops = lower(spec, ver=ver)
        shas[ver] = DveOpSpec(
            name="MUL_SCAN_ANT", opcode=row, uops=uops, rd1_en=_has_src1(spec)
        ).sha(ver)
    op = dvo.DveOp("MUL_SCAN_ANT", spec, subdim=False, uops_sha=shas)
    dvo.OPS.append(op)
    dvo.CUSTOM_DVE_SPECS["MUL_SCAN_ANT"] = spec
    dvo._SUB_OPCODE_FOR_NAME["MUL_SCAN_ANT"] = row
    return op


def _build_launch_a():
    # q,k (cols 0:1024) in fp8 + DoubleRow; v,skip (cols 1024:1600) in bf16.
    nc = bacc.Bacc("TRN2", debug=False, num_devices=NCORES)
    xT8 = nc.dram_tensor("xT8", [KCH * 128, NPAD], FP8, kind="ExternalInput").ap()
    xT = nc.dram_tensor("xT", [KCH * 128, NPAD], BF16, kind="ExternalInput").ap()
    w8 = nc.dram_tensor("w8", [KCH * 128, 1024], FP8, kind="ExternalInput").ap()
    w16 = nc.dram_tensor("w16", [KCH * 128, 576], BF16, kind="ExternalInput").ap()
    bqkvs = nc.dram_tensor("bqkvs", [128, 1600], BF16, kind="ExternalInput").ap()
    qk_out = nc.dram_tensor("qk_out", [NPAD, 1024], BF16, kind="ExternalOutput").ap()
    v_out = nc.dram_tensor("v_out", [NPAD, 512], BF16, kind="ExternalOutput").ap()
    skip_out = nc.dram_tensor("skip_out", [NPAD, OUT_DIM], BF16, kind="ExternalOutput").ap()

    DR = mybir.MatmulPerfMode.DoubleRow

    with tile.TileContext(nc) as tc:
        with (
            tc.tile_pool(name="const", bufs=1) as cpool,
            tc.tile_pool(name="work", bufs=3) as wpool,
            tc.tile_pool(name="psqk", bufs=2, space="PSUM") as pqk,
            tc.tile_pool(name="psvs", bufs=2, space="PSUM") as pvs,
        ):
            # fp8 operands in single tiles (DoubleRow APs span two k-chunks);
            # bf16 x per-k so early matmuls only wait on their own DMA chunk
            x8_s = [cpool.tile([128, 2 * NPAD], FP8, name=f"x8s_{s}")
                    for s in range(KCH // 2)]
            w8_sb = cpool.tile([128, KCH * 1024], FP8)
            x16_k = [cpool.tile([128, NPAD], BF16, name=f"x16_{k}") for k in range(KCH)]
            w16_k = [cpool.tile([128, 576], BF16, name=f"w16_{k}") for k in range(KCH)]
            bqkvs_sb = cpool.tile([128, 1600], BF16)
            # DMA issue order ~ first-consumer order: weights, then x by k
            nc.sync.dma_start(bqkvs_sb[:], bqkvs[:])
            for k in range(KCH):
                nc.sync.dma_start(w8_sb[:, k * 1024:(k + 1) * 1024], w8[ts(k, 128), :])
                nc.sync.dma_start(w16_k[k][:], w16[ts(k, 128), :])
            for s in range(KCH // 2):
                for i in range(2):
                    nc.sync.dma_start(
                        x8_s[s][:, i * NPAD:(i + 1) * NPAD], xT8[ts(2 * s + i, 128), :])
                nc.sync.dma_start(x16_k[2 * s][:], xT[ts(2 * s, 128), :])
                nc.sync.dma_start(x16_k[2 * s + 1][:], xT[ts(2 * s + 1, 128), :])

            w8v = w8_sb.rearrange("p (k n) -> p k n", k=KCH)

            for m in range(TILES):
                qk_ps = pqk.tile([128, 1024], F32, tag="qk")
                vs_ps = pvs.tile([128, 576], F32, tag="vs")
                for s in range(KCH // 2):  # fp8 DoubleRow: 2 k-chunks per pass
                    for n0 in (0, 512):
                        nc.tensor.matmul(
                            qk_ps[:, n0:n0 + 512],
                            lhsT=x8_s[s].rearrange("p (i m) -> p i m", i=2)[:, :, ts(m, 128)],
                            rhs=w8v[:, 2 * s:2 * s + 2, n0:n0 + 512],
                            start=(s == 0),
                            stop=(s == KCH // 2 - 1),
                            perf_mode=DR,
                        )
                for k in range(KCH):  # bf16: v + skip
                    for n0, nw in ((0, 512), (512, 64)):
                        nc.tensor.matmul(
                            vs_ps[:, n0:n0 + nw],
                            lhsT=x16_k[k][:, ts(m, 128)],
                            rhs=w16_k[k][:, n0:n0 + nw],
                            start=(k == 0),
                            stop=(k == KCH - 1),
                        )
                qk_sb = wpool.tile([128, 1024], BF16, tag="qk")
                nc.vector.scalar_tensor_tensor(
                    out=qk_sb[:], in0=qk_ps[:], scalar=1.0 / W_SCALE,
                    in1=bqkvs_sb[:, :1024],
                    op0=mybir.AluOpType.mult, op1=mybir.AluOpType.add,
                )
                v_sb = wpool.tile([128, 512], BF16, tag="v")
                nc.vector.tensor_add(v_sb[:], vs_ps[:, 0:512], bqkvs_sb[:, 1024:1536])
                skip_sb = wpool.tile([128, OUT_DIM], BF16, tag="skip")
                nc.vector.tensor_add(skip_sb[:], vs_ps[:, 512:576], bqkvs_sb[:, 1536:1600])
                nc.sync.dma_start(qk_out[ts(m, 128), :], qk_sb[:])
                nc.sync.dma_start(v_out[ts(m, 128), :], v_sb[:])
                nc.sync.dma_start(skip_out[ts(m, 128), :], skip_sb[:])
    nc.compile()
    return nc


def _build_launch_b(widths):
    """widths: tuple of TILES ints, chunks per dst tile (non-increasing)."""
    mul_scan = _register_mul_scan()
    totch = sum(widths)
    offs = np.zeros(TILES + 1, np.int64)
    offs[1:] = np.cumsum(widths)

    nc = bacc.Bacc("TRN2", debug=False, num_devices=NCORES)
    qg = nc.dram_tensor("qg", [128, totch * HC], FP8, kind="ExternalInput").ap()
    kg = nc.dram_tensor("kg", [128, totch * HC], FP8, kind="ExternalInput").ap()
    vg = nc.dram_tensor("vg", [128, totch * HC], BF16, kind="ExternalInput").ap()
    ind = nc.dram_tensor("ind", [128, totch * 128], FP8, kind="ExternalInput").ap()
    skip = nc.dram_tensor("skip", [TILES, 128, OUT_DIM], BF16, kind="ExternalInput").ap()
    indng = nc.dram_tensor("indng", [TILES, 128, B], FP8, kind="ExternalInput").ap()
    pooled = nc.dram_tensor("pooled", [B, HC], F32, kind="ExternalOutput").ap()
    pskip = nc.dram_tensor("pskip", [B, OUT_DIM], F32, kind="ExternalOutput").ap()

    scale = 1.0 / np.sqrt(np.float32(OUT_DIM))
    W0 = max(widths)
    MAXFW = W0 * HC

    with tile.TileContext(nc) as tc:
        with (
            tc.tile_pool(name="io", bufs=4) as iop,
            tc.tile_pool(name="work", bufs=2) as wp,
            tc.tile_pool(name="psA", bufs=2, space="PSUM") as psA,
            tc.tile_pool(name="psB", bufs=2, space="PSUM") as psB,
            tc.tile_pool(name="psP", bufs=1, space="PSUM") as psP,
            tc.tile_pool(name="psS", bufs=1, space="PSUM") as psS,
            tc.tile_pool(name="outp", bufs=1) as outp,
        ):
            pool_ps = psP.tile([B, HC], F32)
            pskip_ps = psS.tile([B, OUT_DIM], F32)
            for t in range(TILES):
                W = widths[t]
                fw = W * HC
                nb = W * HEADS
                o = int(offs[t])
                qg_sb = iop.tile([128, MAXFW], FP8, tag="qg")
                kg_sb = iop.tile([128, MAXFW], FP8, tag="kg")
                vg_sb = iop.tile([128, MAXFW], BF16, tag="vg")
                ind_sb = iop.tile([128, W0 * 128], FP8, tag="ind")
                skip_sb = iop.tile([128, OUT_DIM], BF16, tag="skip")
                indng_sb = iop.tile([128, B], FP8, tag="indng")
                nc.sync.dma_start(qg_sb[:, :fw], qg[:, o * HC:(o + W) * HC])
                nc.sync.dma_start(kg_sb[:, :fw], kg[:, o * HC:(o + W) * HC])
                nc.sync.dma_start(vg_sb[:, :fw], vg[:, o * HC:(o + W) * HC])
                nc.sync.dma_start(ind_sb[:, :W * 128], ind[:, o * 128:(o + W) * 128])
                nc.sync.dma_start(skip_sb[:], skip[t])
                nc.sync.dma_start(indng_sb[:], indng[t])

                # prefix sums of q*k products along the whole row
                scan_f = wp.tile([128, MAXFW], F32, tag="scan")
                nc.vector._custom_dve(
                    mul_scan, out=scan_f[:, :fw], in0=qg_sb[:, :fw], in1=kg_sb[:, :fw]
                )
                # scores = diffs of prefix at 64-elem boundaries
                rpad = wp.tile([128, W0 * HEADS + 8], F32, tag="rpad")
                nc.vector.memset(rpad[:, 0:1], 0.0)
                nc.vector.tensor_copy(
                    rpad[:, 1:nb + 1],
                    scan_f[:, :fw].rearrange("p (s c) -> p s c", s=nb)[:, :, OUT_DIM - 1],
                )
                s_f = wp.tile([128, W0 * HEADS], F32, tag="s")
                nc.vector.tensor_sub(s_f[:, :nb], rpad[:, 1:nb + 1], rpad[:, 0:nb])
                # w = exp(scale * s), broadcast over the 64 head-channels
                w_bf = wp.tile([128, MAXFW], BF16, tag="w")
                nc.scalar.activation(
                    out=w_bf[:, :fw].rearrange("p (s c) -> p s c", s=nb),
                    in_=s_f[:, :nb].rearrange("p s -> p s ()").to_broadcast(
                        [128, nb, OUT_DIM]),
                    func=mybir.ActivationFunctionType.Exp,
                    scale=float(scale),
                )
                wv = wp.tile([128, MAXFW], BF16, tag="wv")
                nc.vector.tensor_mul(wv[:, :fw], vg_sb[:, :fw], w_bf[:, :fw])

                num_ps = psA.tile([128, HC], F32, tag="num")
                den_ps = psB.tile([128, HEADS], F32, tag="den")
                w4 = w_bf[:, :fw].rearrange("p (ch h c) -> p ch h c", ch=W, h=HEADS)
                for c in range(W):
                    nc.tensor.matmul(
                        num_ps[:], lhsT=ind_sb[:, ts(c, 128)], rhs=wv[:, ts(c, HC)],
                        start=(c == 0), stop=(c == W - 1),
                    )
                    nc.tensor.matmul(
                        den_ps[:], lhsT=ind_sb[:, ts(c, 128)], rhs=w4[:, c, :, 0],
                        start=(c == 0), stop=(c == W - 1),
                    )
                # mh = num / (8*den)  (mean over heads folded into rec)
                rec = wp.tile([128, HEADS], F32, tag="rec")
                nc.vector.tensor_scalar(
                    out=rec[:], in0=den_ps[:],
                    scalar1=float(HEADS), scalar2=1e-6,
                    op0=mybir.AluOpType.mult, op1=mybir.AluOpType.add,
                )
                nc.vector.reciprocal_approx_fast(rec[:], rec[:])
                mh = wp.tile([128, HC], BF16, tag="mh")
                nc.vector.tensor_mul(
                    mh.rearrange("p (h c) -> p h c", h=HEADS),
                    num_ps.rearrange("p (h c) -> p h c", h=HEADS),
                    rec.rearrange("p h -> p h ()").to_broadcast([128, HEADS, OUT_DIM]),
                )
                nc.tensor.matmul(
                    pool_ps[:], lhsT=indng_sb[:], rhs=mh[:],
                    start=(t == 0), stop=(t == TILES - 1),
                )
                nc.tensor.matmul(
                    pskip_ps[:], lhsT=indng_sb[:], rhs=skip_sb[:],
                    start=(t == 0), stop=(t == TILES - 1),
                )
            pooled_sb = outp.tile([B, HC], F32)
            nc.vector.tensor_copy(pooled_sb[:], pool_ps[:])
            nc.sync.dma_start(pooled[:], pooled_sb[:])
            pskip_sb = outp.tile([B, OUT_DIM], F32)
            nc.vector.tensor_copy(pskip_sb[:], pskip_ps[:])
            nc.sync.dma_start(pskip[:], pskip_sb[:])
    nc.compile()
    return nc


def _get_a():
    if "A" not in _cache:
        _cache["A"] = _build_launch_a()
    return _cache["A"]


def _get_b(widths):
    key = ("B", widths)
    if key not in _cache:
        _cache[key] = _build_launch_b(widths)
    return _cache[key]


def _pack_nodes(deg_dst):
    """Assign nodes to (core, tile, idx) balancing edge counts.

    Returns node2slot [N] -> global slot (core*NPAD + tile*128 + idx) and
    the per-tile chunk widths (shared across cores, non-increasing).
    """
    order = np.argsort(-deg_dst, kind="stable")
    # serpentine across cores to equalize per-core edge totals
    ranks = np.arange(N)
    blk, j = ranks // NCORES, ranks % NCORES
    core_for_rank = np.where(blk % 2 == 0, j, NCORES - 1 - j)
    node2slot = np.zeros(N, np.int64)
    widths_per_core = []
    for c in range(NCORES):
        nodes_c = order[core_for_rank == c]  # degree-descending
        # greedy: put each node in the least-loaded tile with room
        heap = [(0, 0, t) for t in range(TILES)]  # (edge load, node count, tile)
        heapq.heapify(heap)
        assign_tile = np.zeros(NPC, np.int64)
        loads = np.zeros(TILES, np.int64)
        cnts = np.zeros(TILES, np.int64)
        for i, n in enumerate(nodes_c):
            load, cnt, t2 = heapq.heappop(heap)
            assign_tile[i] = t2
            load += int(deg_dst[n])
            cnt += 1
            loads[t2] = load
            cnts[t2] = cnt
            if cnt < 128:
                heapq.heappush(heap, (load, cnt, t2))
        w_c = np.maximum(np.ceil(loads / 128).astype(np.int64), 1)
        # relabel tiles so widths are non-increasing (aligned across cores)
        tile_order = np.argsort(-(w_c * 1000000 + loads), kind="stable")
        rank_of_tile = np.empty(TILES, np.int64)
        rank_of_tile[tile_order] = np.arange(TILES)
        new_tile = rank_of_tile[assign_tile]
        # idx within tile = arrival order
        idx_within = np.zeros(NPC, np.int64)
        cnt2 = np.zeros(TILES, np.int64)
        for i in range(NPC):
            t2 = new_tile[i]
            idx_within[i] = cnt2[t2]
            cnt2[t2] += 1
        node2slot[nodes_c] = c * NPAD + new_tile * 128 + idx_within
        widths_per_core.append(w_c[tile_order])
    widths = np.max(np.stack(widths_per_core), axis=0)
    widths = np.minimum(widths, MAXCH)
    return node2slot, tuple(int(w) for w in widths)


LAST_TRACE_PATH = {}


def _ensure_hook_shim():
    import sys
    import types

    if "antenv.axon_hooks" in sys.modules:
        return
    mod = types.ModuleType("antenv.axon_hooks")
    holder = [None]
    mod.set_axon_ntff_profile_hook = lambda h: holder.__setitem__(0, h)
    mod.get_axon_ntff_profile_hook = lambda: holder[0]
    sys.modules["antenv.axon_hooks"] = mod
    import antenv

    antenv.axon_hooks = mod
    from trn_agent_boot.trn_boot import _ntff_profile_via_ctypes

    mod.set_axon_ntff_profile_hook(
        _ntff_profile_via_ctypes("/opt/axon/libaxon_pjrt.so")
    )


def _run(nc, in_maps, label):
    if not TRACE:
        res = bass_utils.run_bass_kernel_spmd(nc, in_maps, list(range(NCORES)))
        return res.results

    import glob
    import os
    import tempfile

    from concourse import bass2jax
    from concourse._compat import FishPath
    import gauge.profiler

    _ensure_hook_shim()
    import antenv.axon_hooks as hooks

    tmpdir = tempfile.mkdtemp(prefix=f"bass_{label}_")
    with hooks.get_axon_ntff_profile_hook()(tmpdir, [0]):
        results = bass2jax.run_bass_via_pjrt(nc, in_maps, n_cores=NCORES)
    exec_ns = None
    try:
        ntffs = glob.glob(os.path.join(tmpdir, "*_body*.ntff"))
        if ntffs:
            profile = gauge.profiler.Profile(
                profile_path=FishPath(tmpdir),
                kernel_dev_mode=True,
                profile_on_exit=False,
                bass_kernel=nc.m,
                offline_processing=True,
                fname="*_body*",
            )
            prs = profile.to_perfetto(model_index=(0,))
            if prs:
                exec_ns = max(p.exec_time_ns for p in prs)
                LAST_TRACE_PATH[label] = (tmpdir, [p.trace_path for p in prs])
        else:
            print(f"[{label}] no ntff files in {tmpdir}: {os.listdir(tmpdir)}")
    except Exception as e:  # profiling must never break the run
        print(f"[{label}] profile processing failed: {type(e).__name__}: {e}")
    LAST_EXEC_NS[label] = exec_ns
    return results


def kernel(x, edge_index, batch, W_emb, b_emb, Wq, bq, Wk, bk, Wv, bv, Wskip, bskip):
    x = np.asarray(x, np.float32)
    edge_index = np.asarray(edge_index)
    batch_np = np.asarray(batch)
    src = np.asarray(edge_index[0], np.int64)
    dst = np.asarray(edge_index[1], np.int64)

    deg = np.bincount(dst, minlength=N)
    node2slot, widths = _pack_nodes(deg)
    totch = sum(widths)
    ncA = _get_a()
    ncB = _get_b(widths)

    # ---- host prep for launch A: fold W_emb/b_emb into the qkv/skip weights ----
    wcat = np.concatenate(
        [np.asarray(Wq, np.float32), np.asarray(Wk, np.float32),
         np.asarray(Wv, np.float32), np.asarray(Wskip, np.float32)], axis=1
    )  # [768, 1600]
    bcat = np.concatenate(
        [np.asarray(bq, np.float32), np.asarray(bk, np.float32),
         np.asarray(bv, np.float32), np.asarray(bskip, np.float32)]
    )  # [1600]
    wemb_f = np.asarray(W_emb, np.float32)
    bemb_f = np.asarray(b_emb, np.float32)
    wqkvs_f = wemb_f @ wcat                          # [768, 1600]
    w8 = np.ascontiguousarray(wqkvs_f[:, :1024] * W_SCALE).astype(NP_FP8)
    w16 = np.ascontiguousarray(wqkvs_f[:, 1024:1600]).astype(NP_BF16)
    bqkvs = (bemb_f @ wcat + bcat).astype(np.float32)
    bqkvs_rep = np.broadcast_to(bqkvs.astype(NP_BF16), (128, 1600)).copy()

    xpad = np.zeros((NCORES * NPAD, IN_DIM), np.float32)
    xpad[node2slot] = x
    in_maps_a = []
    for c in range(NCORES):
        xT = np.ascontiguousarray(xpad[c * NPAD:(c + 1) * NPAD].T)  # [768, 6272]
        in_maps_a.append({
            "xT8": xT.astype(NP_FP8), "xT": xT.astype(NP_BF16),
            "w8": w8, "w16": w16, "bqkvs": bqkvs_rep,
        })
    res_a = _run(ncA, in_maps_a, "A")

    # ---- host mid: slot-ordered Q,K,V and edge-sorted gathers ----
    QK8 = np.concatenate([res_a[c]["qk_out"] for c in range(NCORES)]).astype(NP_FP8)
    V = np.concatenate([res_a[c]["v_out"] for c in range(NCORES)])  # bf16 [8*NPAD,512]

    dslot = node2slot[dst]
    tile_g = dslot // 128  # global tile id: core*TILES + tile
    dloc = dslot % 128
    order = np.argsort(tile_g, kind="stable")
    tg_s, src_s, dloc_s, dslot_s = tile_g[order], src[order], dloc[order], dslot[order]
    ntile = NCORES * TILES
    counts = np.bincount(tg_s, minlength=ntile)
    wid_g = np.tile(np.asarray(widths, np.int64), NCORES)
    cap_g = wid_g * 128
    if np.any(counts > cap_g):
        raise RuntimeError("tile capacity exceeded after packing")
    # edge-slot base per global tile in the variable-width flat layout
    tots = totch * 128  # slots per core
    cumw = np.zeros(TILES + 1, np.int64)
    cumw[1:] = np.cumsum(np.asarray(widths, np.int64))
    ebase = (tg_s // TILES) * tots + cumw[tg_s % TILES] * 128
    starts = np.zeros(ntile, np.int64)
    starts[1:] = np.cumsum(counts)[:-1]
    pos = np.arange(E) - starts[tg_s]
    rows = ebase + pos

    nslot_t = NCORES * tots
    srcslot_pad = np.zeros(nslot_t, np.int64)
    srcslot_pad[rows] = node2slot[src_s]
    dloc_pad = np.full(nslot_t, -1, np.int64)
    dloc_pad[rows] = dloc_s
    dslot_pad = np.zeros(nslot_t, np.int64)
    dslot_pad[rows] = dslot_s

    def tileize(a):  # per core [tots, D] -> [128, totch*D], chunk-major cols
        d = a.shape[1]
        return np.ascontiguousarray(
            a.reshape(totch, 128, d).transpose(1, 0, 2).reshape(128, totch * d)
        )

    qg_f = QK8[dslot_pad, 0:512].reshape(NCORES, tots, 512)
    kg_f = QK8[srcslot_pad, 512:1024].reshape(NCORES, tots, 512)
    vg_f = V[srcslot_pad].reshape(NCORES, tots, 512)
    ind_f = (dloc_pad[:, None] == np.arange(128)[None, :]).astype(NP_FP8).reshape(
        NCORES, tots, 128)

    batch_pad = np.full(NCORES * NPAD, -1, np.int64)
    batch_pad[node2slot] = batch_np
    indng = (batch_pad[:, None] == np.arange(B)[None, :]).astype(NP_FP8)
    indng = indng.reshape(NCORES, TILES, 128, B)

    in_maps_b = []
    for c in range(NCORES):
        in_maps_b.append({
            "qg": tileize(qg_f[c]), "kg": tileize(kg_f[c]),
            "vg": tileize(vg_f[c]), "ind": tileize(ind_f[c]),
            "skip": np.ascontiguousarray(
                res_a[c]["skip_out"].reshape(TILES, 128, OUT_DIM)),
            "indng": indng[c],
        })
    res_b = _run(ncB, in_maps_b, "B")

    pooled = np.zeros((B, OUT_DIM), np.float64)
    for c in range(NCORES):
        ph = res_b[c]["pooled"].astype(np.float64)  # [B, 512]
        pooled += ph.reshape(B, HEADS, OUT_DIM).sum(axis=1)
        pooled += res_b[c]["pskip"].astype(np.float64)
    cnt = np.bincount(batch_np, minlength=B).astype(np.float64)
    pooled /= np.maximum(cnt, 1.0)[:, None]
    return pooled.astype(np.float32)
